# revision 14
# baseline (speedup 1.0000x reference)
"""Multi-head self-attention with RoPE (B=4, S=2048, D=768, H=12, DH=64),
distributed over 8 NeuronCores: batch x head-group sharding.

Core c handles batch b = c//2, head group g = c%2 (6 heads each).
Per-core kernel computes qkv projection, RoPE, causal attention and the
*partial* output projection for its 6 heads; the host sums the two
partials per batch (the "all-reduce after output proj" step done on host
during unsharding).

Layout notes (all on-chip tensors are [partition, free]):
 - x is fed transposed (xt [768, 2048]) so both qk^T ([e, s]) and v
   ([s, e]) orientations come straight out of the tensor engine.
 - RoPE pair-deinterleaving is folded into the row order of the qkv
   weight shard, so the rotation is pure block ops (no strided access):
     QT0 = x1(even dims) of heads 0..3   QT1 = x2(odd dims) of heads 0..3
     QT2 = [x1 h4 | x2 h4 | x1 h5 | x2 h5]
   (same for K). Scores contraction then takes q/k of head h as one
   K=64 slice (heads 4,5) or two accumulating K=32 slices (heads 0..3).
 - 1/sqrt(DH) is folded into the q weights on the host.
 - scores are computed transposed ([kpos, qpos]) so softmax(exp)@V needs
   no transposes; row sums come from 64 "ones" columns appended to V,
   which replicate the sums across partitions 64..127 for a
   broadcast-free normalization.
"""

import numpy as np

B, S, D = 4, 2048, 768
H, DH, HALF = 12, 64, 32
NH = 6  # heads per core
THETA = 10000.0
NCORES = 8
P = 128
NQ = 4  # q quarters of 512
QW = 512


def _head_rows(base, h, parity):
    # rows of w_qkv for head h, even (parity=0) or odd dims
    return [base + h * DH + 2 * i + parity for i in range(HALF)]


def _qk_row_order(g):
    """Row order (into w_qkv) for the QK part of the weight shard."""
    hs = [g * NH + i for i in range(NH)]
    rows = []
    for base in (0, D):  # Q block, K block
        # T0: x1 of heads 0..3
        for hl in range(4):
            rows += _head_rows(base, hs[hl], 0)
        # T1: x2 of heads 0..3
        for hl in range(4):
            rows += _head_rows(base, hs[hl], 1)
        # T2: [x1 h4 | x2 h4 | x1 h5 | x2 h5]
        for hl in (4, 5):
            rows += _head_rows(base, hs[hl], 0)
            rows += _head_rows(base, hs[hl], 1)
    return rows


def _v_row_order(g):
    hs = [g * NH + i for i in range(NH)]
    rows = []
    for h in hs:
        rows += [2 * D + h * DH + j for j in range(DH)]
    return rows


def _cos_sin_rep():
    inv_freq = THETA ** (-np.arange(HALF, dtype=np.float64) * 2.0 / DH)
    ang = np.arange(S, dtype=np.float64)[:, None] * inv_freq[None, :]  # [S, 32]
    cos = np.cos(ang).astype(np.float32).T  # [32, S]
    sin = np.sin(ang).astype(np.float32).T
    cos_rep = np.tile(cos, (4, 1))  # [128, S]
    sin_rep = np.tile(sin, (4, 1))
    return cos_rep, sin_rep


def _trimask():
    # upper-triangular inclusive: m[p, c] = 1 if c >= p
    idx = np.arange(P)
    return (idx[None, :] >= idx[:, None]).astype(np.float32)


def make_core_inputs(x, w_qkv, w_out):
    """Shard the full inputs into the 8 per-core input maps."""
    x = np.asarray(x, dtype=np.float32)
    w_qkv = np.asarray(w_qkv, dtype=np.float32)
    w_out = np.asarray(w_out, dtype=np.float32)
    cos_rep, sin_rep = _cos_sin_rep()
    tri = _trimask()
    in_maps = []
    for c in range(NCORES):
        b, g = c // 2, c % 2
        wqk = w_qkv[_qk_row_order(g), :].copy()  # [768, 768]
        wqk[:384, :] *= 1.0 / np.sqrt(DH)  # fold score scale into q
        wv = w_qkv[_v_row_order(g), :]  # [384, 768]
        wo = w_out[:, g * 384:(g + 1) * 384]  # [768, 384]
        in_maps.append({
            "xt": np.ascontiguousarray(x[b].T),            # [768, 2048]
            "wqkt": np.ascontiguousarray(wqk.T),           # [768, 768]
            "wvt": np.ascontiguousarray(wv.T),             # [768, 384]
            "wo": np.ascontiguousarray(wo.T),              # [384, 768]
            "cosr": cos_rep,                               # [128, 2048]
            "sinr": sin_rep,                               # [128, 2048]
            "tri": tri,                                    # [128, 128]
        })
    return in_maps


def build_nc(split=True):
    import concourse.bass as bass
    import concourse.mybir as mybir
    import concourse.tile as tile

    f32 = mybir.dt.float32
    f32r = mybir.dt.float32r
    EXP = mybir.ActivationFunctionType.Exp


    nc = bass.Bass()
    xt_d = nc.declare_dram_parameter("xt", [D, S], f32r, isOutput=False)
    wqkt_d = nc.declare_dram_parameter("wqkt", [D, D], f32r, isOutput=False)
    wvt_d = nc.declare_dram_parameter("wvt", [D, 384], f32r, isOutput=False)
    wo_d = nc.declare_dram_parameter("wo", [384, D], f32r, isOutput=False)
    cos_d = nc.declare_dram_parameter("cosr", [P, S], f32, isOutput=False)
    sin_d = nc.declare_dram_parameter("sinr", [P, S], f32, isOutput=False)
    tri_d = nc.declare_dram_parameter("tri", [P, P], f32, isOutput=False)
    out_d = nc.declare_dram_parameter("out", [S, D], f32, isOutput=True)

    KD = D // P  # 6 contraction tiles over d
    SM = S // P  # 16 sequence tiles of 128

    with tile.TileContext(nc) as tc:
        # ---- persistent pools ----------------------------------------
        with (
            tc.tile_pool(name="rotqk", bufs=1) as rot_pool,      # 48K/p
            tc.tile_pool(name="vext", bufs=1) as vext_pool,     # 48K/p
            tc.tile_pool(name="smallc", bufs=1) as small_pool,   # tri
        ):
            tri_t = small_pool.tile([P, P], f32, tag="tri")
            nc.sync.dma_start(tri_t[:], tri_d[:])

            vext = [vext_pool.tile([P, NH * P], f32r, name=f"vext{i}", tag=f"vext{i}") for i in range(SM)]

            # ---- phase 1: QK^T = wqk @ x^T, then RoPE ---------------
            with tc.tile_pool(name="xt", bufs=1) as xt_pool:
                xt = [xt_pool.tile([P, S], f32r, name=f"xt{i}", tag=f"xt{i}") for i in range(KD)]
                for kd in range(KD):
                    nc.sync.dma_start(xt[kd][:], xt_d[kd * P:(kd + 1) * P, :])
                with (
                    tc.tile_pool(name="csin", bufs=1) as cs_pool,
                    tc.tile_pool(name="wqk", bufs=1) as wqk_pool,
                    tc.tile_pool(name="qkps", bufs=2, space="PSUM") as qk_psum,
                    tc.tile_pool(name="rtmp", bufs=2) as tmp_pool,
                ):
                    cos_t = cs_pool.tile([P, S], f32, tag="cos")
                    sin_t = cs_pool.tile([P, S], f32, tag="sin")
                    nc.sync.dma_start(cos_t[:], cos_d[:])
                    nc.sync.dma_start(sin_t[:], sin_d[:])
                    wqk = [wqk_pool.tile([P, D], f32r, name=f"wqk{i}", tag=f"wqk{i}") for i in range(KD)]
                    for kd in range(KD):
                        nc.sync.dma_start(
                            wqk[kd][:], wqkt_d[kd * P:(kd + 1) * P, :])

                    raw = []
                    for m in range(KD):
                        ps = qk_psum.tile([P, S], f32, name="qkps", tag="qkps")
                        for n in range(S // 512):
                            ncol = slice(n * 512, (n + 1) * 512)
                            for kd in range(KD):
                                nc.tensor.matmul(
                                    ps[:, ncol],
                                    wqk[kd][:, m * P:(m + 1) * P],
                                    xt[kd][:, ncol],
                                    start=(kd == 0), stop=(kd == KD - 1),
                                )
                        rt = rot_pool.tile([P, S], f32r, name=f"rot{m}", tag=f"rot{m}")
                        nc.scalar.copy(rt[:], ps[:])
                        raw.append(rt)

                    # RoPE in place on the raw tiles.
                    def rope_pair(x1t, x2t):
                        t1 = tmp_pool.tile([P, S], f32, name="rtmp", tag="rtmp")
                        t2 = tmp_pool.tile([P, S], f32, name="rtmp", tag="rtmp")
                        nc.vector.tensor_mul(t1[:], x2t[:], sin_t[:])
                        nc.vector.tensor_mul(t2[:], x1t[:], sin_t[:])
                        nc.vector.tensor_mul(x1t[:], x1t[:], cos_t[:])
                        nc.vector.tensor_sub(x1t[:], x1t[:], t1[:])
                        nc.vector.tensor_mul(x2t[:], x2t[:], cos_t[:])
                        nc.vector.tensor_add(x2t[:], x2t[:], t2[:])

                    def rope_mixed(mt):
                        # [x1 hA | x2 hA | x1 hB | x2 hB] in 32-row blocks
                        t = tmp_pool.tile([P, S], f32, name="rtmp", tag="rtmp")
                        for blk in (0, 64):
                            a = slice(blk, blk + 32)
                            bsl = slice(blk + 32, blk + 64)
                            nc.vector.tensor_mul(t[a], mt[bsl], sin_t[bsl])
                            nc.vector.tensor_mul(t[bsl], mt[a], sin_t[a])
                        nc.vector.tensor_mul(mt[:], mt[:], cos_t[:])
                        for blk in (0, 64):
                            a = slice(blk, blk + 32)
                            bsl = slice(blk + 32, blk + 64)
                            nc.vector.tensor_sub(mt[a], mt[a], t[a])
                            nc.vector.tensor_add(mt[bsl], mt[bsl], t[bsl])

                    rope_pair(raw[0], raw[1])   # q heads 0..3
                    rope_mixed(raw[2])          # q heads 4,5
                    rope_pair(raw[3], raw[4])   # k heads 0..3
                    rope_mixed(raw[5])          # k heads 4,5
                    QT0, QT1, QT2, KT0, KT1, KT2 = raw

                # ---- phase 2: V (normal orientation) + vext ----------
                with (
                    tc.tile_pool(name="wv", bufs=1) as wv_pool,
                    tc.tile_pool(name="vps", bufs=2, space="PSUM") as v_psum,
                ):
                    wv = [wv_pool.tile([P, 384], f32r, name=f"wv{i}", tag=f"wv{i}") for i in range(KD)]
                    for kd in range(KD):
                        nc.sync.dma_start(
                            wv[kd][:], wvt_d[kd * P:(kd + 1) * P, :])
                    for sm in range(SM):
                        vx = vext[sm][:].rearrange("p (h c) -> p h c", c=P)
                        nc.vector.memset(vx[:, :, DH:P].bitcast(f32), 1.0)
                        vps = v_psum.tile([P, 384], f32, name="vps", tag="vps")
                        for kd in range(KD):
                            nc.tensor.matmul(
                                vps[:],
                                xt[kd][:, sm * P:(sm + 1) * P],
                                wv[kd][:],
                                start=(kd == 0), stop=(kd == KD - 1),
                            )
                        nc.scalar.copy(
                            vx[:, :, 0:DH],
                            vps[:].rearrange("p (h c) -> p h c", c=DH),
                        )

            # ---- phase 3: attention ---------------------------------
            # per head: list of (kT_tile, qT_tile, base_partition, kwidth)
            def head_parts(h):
                if h < 4:
                    return [(KT0, QT0, 32 * h, 32), (KT1, QT1, 32 * h, 32)]
                return [(KT2, QT2, 64 * (h - 4), 64)]

            with tc.tile_pool(name="attnT", bufs=1) as attnT_pool:
              attnT = [attnT_pool.tile([P, S], f32r, name=f"attnT{i}", tag=f"attnT{i}") for i in range(3)]
              with (
                tc.tile_pool(name="sps", bufs=4, space="PSUM") as s_psum,
                tc.tile_pool(name="aps", bufs=2, space="PSUM") as a_psum,
                tc.tile_pool(name="expt", bufs=6) as exp_pool,
                tc.tile_pool(name="rcp", bufs=4) as rcp_pool,
            ):
                for hp in range(3):
                    pair = (2 * hp, 2 * hp + 1)
                    for qq in range(NQ):
                        acc = {h: a_psum.tile([P, QW], f32, name="aps", tag="aps") for h in pair}
                        nkj = 4 * qq + 4
                        for kj in range(nkj):
                            m = kj - 4 * qq  # >=0 on diagonal blocks
                            coff = 128 * m if m > 0 else 0
                            cols = slice(coff, QW)
                            qcols = slice(qq * QW + coff, (qq + 1) * QW)
                            ksl = slice(kj * P, (kj + 1) * P)
                            sco = {}
                            parts = {h: head_parts(h) for h in pair}
                            npart = len(parts[pair[0]])
                            for h in pair:
                                sco[h] = s_psum.tile([P, QW], f32, name="sps", tag="sps")
                            for pi in range(npart):
                                for h in pair:
                                    kt, qt, bp, kw = parts[h][pi]
                                    nc.tensor.matmul(
                                        sco[h][:, cols],
                                        kt[bp:bp + kw, ksl],
                                        qt[bp:bp + kw, qcols],
                                        start=(pi == 0),
                                        stop=(pi == npart - 1),
                                        tile_position=(bp, 0),
                                    )
                            for h in pair:
                                ext = exp_pool.tile([P, QW], f32r, name="expt", tag="expt")
                                nc.scalar.activation(
                                    ext[:, cols], sco[h][:, cols], EXP)
                                if m >= 0:
                                    nc.vector.tensor_mul(
                                        ext[:, coff:coff + P],
                                        ext[:, coff:coff + P],
                                        tri_t[:],
                                    )
                                nc.tensor.matmul(
                                    acc[h][:, cols],
                                    vext[kj][:, h * P:(h + 1) * P],
                                    ext[:, cols],
                                    start=(kj == 0), stop=(kj == nkj - 1),
                                    skip_group_check=True,
                                )
                        for h in pair:
                            rc = rcp_pool.tile([DH, QW], f32, name="rcp", tag="rcp")
                            nc.vector.reciprocal(rc[:], acc[h][DH:P, :])
                            dst = attnT[h // 2][
                                DH * (h % 2):DH * (h % 2) + DH,
                                qq * QW:(qq + 1) * QW,
                            ]
                            nc.vector.tensor_mul(dst, acc[h][0:DH, :], rc[:])

              # ---- phase 4: output projection -----------------------
              with (
                tc.tile_pool(name="wo", bufs=1) as wo_pool,
                tc.tile_pool(name="ops", bufs=2, space="PSUM") as o_psum,
                tc.tile_pool(name="osb", bufs=3) as o_sbuf,
              ):
                wo = [wo_pool.tile([P, D], f32r, name=f"wo{i}", tag=f"wo{i}") for i in range(3)]
                for kd in range(3):
                    nc.sync.dma_start(wo[kd][:], wo_d[kd * P:(kd + 1) * P, :])
                for sm in range(SM):
                    ops = o_psum.tile([P, D], f32, name="ops", tag="ops")
                    ssl = slice(sm * P, (sm + 1) * P)
                    for kd in range(3):
                        nc.tensor.matmul(
                            ops[:, 0:512], attnT[kd][:, ssl],
                            wo[kd][:, 0:512],
                            start=(kd == 0), stop=(kd == 2),
                        )
                        nc.tensor.matmul(
                            ops[:, 512:D], attnT[kd][:, ssl],
                            wo[kd][:, 512:D],
                            start=(kd == 0), stop=(kd == 2),
                        )
                    osb = o_sbuf.tile([P, D], f32, name="osb", tag="osb")
                    nc.scalar.copy(osb[:], ops[:])
                    nc.sync.dma_start(out_d[ssl, :], osb[:])

    if split:
        _split_waits(nc)
    return nc


def _split_waits(nc, keep=1, per_nop=1):
    """Walrus TPB engine instructions support at most 2 sync commands
    (waits + updates).  Tile can emit more waits than that; spill the
    excess onto preceding same-engine NoOps (sequencer-level waits)."""
    import concourse.mybir as mybir

    n = 0
    for f in nc.m.functions:
        for bb in f.blocks:
            out = []
            changed = False
            for inst in bb.instructions:
                si = getattr(inst, "sync_info", None)
                if (
                    si is not None
                    and si.on_wait
                    and not isinstance(inst, mybir.InstEventSemaphore)
                ):
                    nu = len(si.on_update or [])
                    budget = max(0, min(keep, 2 - nu))
                    waits = list(si.on_wait)
                    if len(waits) > budget:
                        excess = waits[budget:]
                        while excess:
                            chunk, excess = excess[:per_nop], excess[per_nop:]
                            nop = mybir.InstNoOp(
                                name=f"wsplit_{n}", engine=inst.engine,
                                ins=[], outs=[])
                            n += 1
                            nop.sync_info = mybir.SyncInfo(
                                on_wait=chunk, on_update=[])
                            out.append(nop)
                        inst.sync_info = mybir.SyncInfo(
                            on_wait=waits[:budget],
                            on_update=list(si.on_update or []))
                        changed = True
                out.append(inst)
            if changed:
                bb.instructions = out
    return nc


_NC_CACHE = None


def get_nc():
    global _NC_CACHE
    if _NC_CACHE is None:
        _NC_CACHE = build_nc()
    return _NC_CACHE


def kernel(x, w_qkv, w_out):
    from concourse.bass_utils import run_bass_kernel_spmd

    in_maps = make_core_inputs(x, w_qkv, w_out)
    res = run_bass_kernel_spmd(get_nc(), in_maps, core_ids=list(range(NCORES)))
    out = np.empty((B, S, D), dtype=np.float32)
    for b in range(B):
        out[b] = res.results[2 * b]["out"] + res.results[2 * b + 1]["out"]
    return out


# revision 19
# speedup vs baseline: 1.0071x; 1.0071x over previous
"""Multi-head self-attention with RoPE (B=4, S=2048, D=768, H=12, DH=64),
distributed over 8 NeuronCores: batch x head-group sharding.

Core c handles batch b = c//2, head group g = c%2 (6 heads each).
Per-core kernel computes the qkv projection, RoPE, causal attention and
the *partial* output projection for its 6 heads; the host sums the two
partials per batch (the "all-reduce after output proj" done on host
during unsharding).

Layout notes (all on-chip tensors are [partition, free]):
 - x is fed transposed (xt [768, 2048]) so both qk^T ([e, s]) and v
   ([s, e]) orientations come straight out of the tensor engine.
 - RoPE pair-deinterleaving is folded into the row order of the qkv
   weight shard.  Uniform per-tile layout (tile t = head pair):
     [x1 h_a (32) | x2 h_a (32) | x1 h_b (32) | x2 h_b (32)]
   so each head's 64 dims are one contiguous K=64 slice at base
   partition 0 or 64, and the two heads of a pair run as concurrent
   row-group matmuls.  The rotation's partner-swap is a 0/1 permutation
   matmul on the tensor engine (exact), then 3 full-width vector ops per
   tile, with the sign pattern baked into the host sin_pm constant.
 - 1/sqrt(DH) is folded into the q weights on the host.
 - scores are computed transposed ([kpos, qpos]) so softmax(exp)@V needs
   no transposes; row sums come from 64 "ones" columns appended to V,
   which replicate the sums across partitions 64..127 for a
   broadcast-free normalization.
 - float32r end-to-end on the matmul path (full-rate fp32 on the PE).
 - a post-pass spills sync waits >1 per engine instruction onto
   preceding NoOps (walrus limit: 1 wait + 1 update per instruction).
"""

import numpy as np

B, S, D = 4, 2048, 768
H, DH, HALF = 12, 64, 32
NH = 6  # heads per core
THETA = 10000.0
NCORES = 8
P = 128
NQ = 4  # q quarters of 512
QW = 512


def _head_rows(base, h, parity):
    return [base + h * DH + 2 * i + parity for i in range(HALF)]


def _qk_row_order(g):
    """Row order (into w_qkv) for the QK part of the weight shard.

    M-tile order: q_pair0, k_pair0, q_pair1, k_pair1, q_pair2, k_pair2;
    each tile is [x1 h_a | x2 h_a | x1 h_b | x2 h_b] (32 rows each).
    """
    hs = [g * NH + i for i in range(NH)]
    rows = []
    for hp in range(3):
        for base in (0, D):  # q tile then k tile
            for hl in (2 * hp, 2 * hp + 1):
                rows += _head_rows(base, hs[hl], 0)
                rows += _head_rows(base, hs[hl], 1)
    return rows


def _v_row_order(g):
    hs = [g * NH + i for i in range(NH)]
    rows = []
    for h in hs:
        rows += [2 * D + h * DH + j for j in range(DH)]
    return rows


def _cos_sin():
    inv_freq = THETA ** (-np.arange(HALF, dtype=np.float64) * 2.0 / DH)
    ang = np.arange(S, dtype=np.float64)[:, None] * inv_freq[None, :]  # [S, 32]
    cos = np.cos(ang).astype(np.float32).T  # [32, S]
    sin = np.sin(ang).astype(np.float32).T
    cos_rep = np.tile(cos, (4, 1))  # [128, S] (every 32-row block: cos)
    # sin with sign baked in: x1 slots (p%64 < 32) get -sin, x2 slots +sin
    sin_pm = np.concatenate([-sin, sin, -sin, sin], axis=0)  # [128, S]
    return cos_rep, sin_pm


def _perm():
    # swap-partner permutation: p%64<32 -> p+32 else p-32; Perm[k,p]=1 iff
    # k=partner(p) (symmetric involution).  lhsT for out = Perm @ rt.
    pm = np.zeros((P, P), np.float32)
    for p in range(P):
        partner = p + 32 if (p % 64) < 32 else p - 32
        pm[partner, p] = 1.0
    return pm


def _trimasks():
    idx = np.arange(P)
    tri = (idx[None, :] >= idx[:, None]).astype(np.float32)  # [128, 128]
    tri2 = np.concatenate([np.zeros((P, P), np.float32), tri], axis=1)
    return tri, tri2  # [128, 128], [128, 256]


def make_core_inputs(x, w_qkv, w_out):
    """Shard the full inputs into the 8 per-core input maps."""
    x = np.asarray(x, dtype=np.float32)
    w_qkv = np.asarray(w_qkv, dtype=np.float32)
    w_out = np.asarray(w_out, dtype=np.float32)
    cos_rep, sin_pm = _cos_sin()
    tri, tri2 = _trimasks()
    perm = _perm()
    qrows = set()
    for hp in range(3):
        base = hp * 256
        qrows.update(range(base, base + 128))
    in_maps = []
    for c in range(NCORES):
        b, g = c // 2, c % 2
        order = _qk_row_order(g)
        wqk = w_qkv[order, :].copy()  # [768, 768]
        scale = 1.0 / np.sqrt(DH)
        for i in sorted(qrows):
            wqk[i, :] *= scale
        wv = w_qkv[_v_row_order(g), :]  # [384, 768]
        wo = w_out[:, g * 384:(g + 1) * 384]  # [768, 384]
        in_maps.append({
            "xt": np.ascontiguousarray(x[b].T),            # [768, 2048]
            "wqkt": np.ascontiguousarray(wqk.T),           # [768, 768]
            "wvt": np.ascontiguousarray(wv.T),             # [768, 384]
            "wo": np.ascontiguousarray(wo.T),              # [384, 768]
            "cosr": cos_rep,                               # [128, 2048]
            "sinpm": sin_pm,                               # [128, 2048]
            "tri": tri,                                    # [128, 128]
            "tri2": tri2,                                  # [128, 256]
            "perm": perm,                                  # [128, 128]
        })
    return in_maps


def build_nc(split=True):
    import concourse.bass as bass
    import concourse.mybir as mybir
    import concourse.tile as tile

    f32 = mybir.dt.float32
    f32r = mybir.dt.float32r
    EXP = mybir.ActivationFunctionType.Exp

    nc = bass.Bass()
    xt_d = nc.declare_dram_parameter("xt", [D, S], f32r, isOutput=False)
    wqkt_d = nc.declare_dram_parameter("wqkt", [D, D], f32r, isOutput=False)
    wvt_d = nc.declare_dram_parameter("wvt", [D, 384], f32r, isOutput=False)
    wo_d = nc.declare_dram_parameter("wo", [384, D], f32r, isOutput=False)
    cos_d = nc.declare_dram_parameter("cosr", [P, S], f32, isOutput=False)
    sin_d = nc.declare_dram_parameter("sinpm", [P, S], f32, isOutput=False)
    tri_d = nc.declare_dram_parameter("tri", [P, P], f32r, isOutput=False)
    tri2_d = nc.declare_dram_parameter("tri2", [P, 256], f32r, isOutput=False)
    perm_d = nc.declare_dram_parameter("perm", [P, P], f32r, isOutput=False)
    out_d = nc.declare_dram_parameter("out", [S, D], f32, isOutput=True)

    KD = D // P  # 6 contraction tiles over d
    SM = S // P  # 16 sequence tiles of 128

    with tile.TileContext(nc) as tc:
        with (
            tc.tile_pool(name="rotqk", bufs=1) as rot_pool,     # 48K/p
            tc.tile_pool(name="vext", bufs=1) as vext_pool,     # 48K/p
            tc.tile_pool(name="smallc", bufs=1) as small_pool,  # tri, tri2
            tc.tile_pool(name="xt", bufs=1) as xt_pool,
            tc.tile_pool(name="expt", bufs=3) as exp_pool,
            tc.tile_pool(name="rcp", bufs=2) as rcp_pool,
        ):
            tri_t = small_pool.tile([P, P], f32r, tag="tri")
            tri2_t = small_pool.tile([P, 256], f32r, tag="tri2")
            perm_t = small_pool.tile([P, P], f32r, tag="perm")
            nc.sync.dma_start(tri_t[:], tri_d[:])
            nc.sync.dma_start(tri2_t[:], tri2_d[:])
            nc.sync.dma_start(perm_t[:], perm_d[:])

            vext = [vext_pool.tile([P, NH * P], f32r, name=f"vext{i}",
                                   tag=f"vext{i}") for i in range(SM)]
            xt = [xt_pool.tile([P, S], f32r, name=f"xt{i}", tag=f"xt{i}")
                  for i in range(KD)]
            for kd in range(KD):
                nc.sync.dma_start(xt[kd][:], xt_d[kd * P:(kd + 1) * P, :])

            # ---- phase 0: V (normal orientation) + vext -------------
            with (
                tc.tile_pool(name="wv", bufs=1) as wv_pool,
                tc.tile_pool(name="vps", bufs=2, space="PSUM") as v_psum,
            ):
                wv = [wv_pool.tile([P, 384], f32r, name=f"wv{i}",
                                   tag=f"wv{i}") for i in range(KD)]
                for kd in range(KD):
                    nc.sync.dma_start(wv[kd][:], wvt_d[kd * P:(kd + 1) * P, :])
                for sm in range(SM):
                    vx = vext[sm][:].rearrange("p (h c) -> p h c", c=P)
                    nc.vector.memset(vx[:, :, DH:P].bitcast(f32), 1.0)
                    vps = v_psum.tile([P, 384], f32, name="vps", tag="vps")
                    for kd in range(KD):
                        nc.tensor.matmul(
                            vps[:],
                            xt[kd][:, sm * P:(sm + 1) * P],
                            wv[kd][:],
                            start=(kd == 0), stop=(kd == KD - 1),
                        )
                    nc.vector.tensor_copy(
                        vx[:, :, 0:DH],
                        vps[:].rearrange("p (h c) -> p h c", c=DH),
                    )

            # ---- phase 1: QK^T = wqk @ x^T, then RoPE ---------------
            with (
                tc.tile_pool(name="csin", bufs=1) as cs_pool,
                tc.tile_pool(name="wqk", bufs=1) as wqk_pool,
                tc.tile_pool(name="qkps", bufs=2, space="PSUM") as qk_psum,
                tc.tile_pool(name="swp", bufs=1) as sw_pool,
            ):
                cos_t = cs_pool.tile([P, S], f32, tag="cos")
                sin_t = cs_pool.tile([P, S], f32, tag="sinpm")
                nc.sync.dma_start(cos_t[:], cos_d[:])
                nc.sync.dma_start(sin_t[:], sin_d[:])
                wqk = [wqk_pool.tile([P, D], f32r, name=f"wqk{i}",
                                     tag=f"wqk{i}") for i in range(KD)]
                for kd in range(KD):
                    nc.sync.dma_start(wqk[kd][:], wqkt_d[kd * P:(kd + 1) * P, :])

                rot = []
                for m in range(KD):
                    ps = qk_psum.tile([P, S], f32, name="qkps", tag="qkps")
                    for n in range(S // 512):
                        ncol = slice(n * 512, (n + 1) * 512)
                        for kd in range(KD):
                            nc.tensor.matmul(
                                ps[:, ncol],
                                wqk[kd][:, m * P:(m + 1) * P],
                                xt[kd][:, ncol],
                                start=(kd == 0), stop=(kd == KD - 1),
                            )
                    rt = rot_pool.tile([P, S], f32r, name=f"rot{m}",
                                       tag=f"rot{m}")
                    nc.scalar.copy(rt[:], ps[:])
                    # RoPE: rt = rt*cos + (Perm @ rt)*sin_pm, swap done on
                    # the PE via a 0/1 permutation matmul.
                    swps = qk_psum.tile([P, S], f32, name="qkps_sw",
                                        tag="qkps")
                    for n in range(S // 512):
                        ncol = slice(n * 512, (n + 1) * 512)
                        nc.tensor.matmul(swps[:, ncol], perm_t[:],
                                         rt[:, ncol], start=True, stop=True)
                    sw = sw_pool.tile([P, S], f32, name="swp", tag="swp")
                    nc.vector.tensor_mul(sw[:], swps[:], sin_t[:])
                    nc.vector.tensor_mul(rt[:], rt[:], cos_t[:])
                    nc.vector.tensor_add(rt[:], rt[:], sw[:])
                    rot.append(rt)

            # ---- phase 3: attention ---------------------------------
            with tc.tile_pool(name="attnT", bufs=1) as attnT_pool:
              attnT = [attnT_pool.tile([P, S], f32r, name=f"attnT{i}",
                                       tag=f"attnT{i}") for i in range(3)]
              with (
                tc.tile_pool(name="sps", bufs=2, space="PSUM") as s_psum,
                tc.tile_pool(name="aps", bufs=2, space="PSUM") as a_psum,
              ):
                for hp in range(3):
                    QT, KT = rot[2 * hp], rot[2 * hp + 1]
                    for qq in range(NQ):
                        acc = {s: a_psum.tile([P, QW], f32, name="aps",
                                              tag="aps") for s in range(2)}
                        nkj = 4 * qq + 4
                        for kj in range(nkj):
                            m = kj - 4 * qq  # >= 0 on diagonal blocks
                            coff = min(128 * m, 256) if m > 0 else 0
                            w = QW - coff
                            qcols = slice(qq * QW + coff, (qq + 1) * QW)
                            ksl = slice(kj * P, (kj + 1) * P)
                            sco = s_psum.tile([P, 2 * QW], f32, name="sps",
                                              tag="sps")
                            for s in range(2):
                                bp = 64 * s
                                nc.tensor.matmul(
                                    sco[:, s * QW + coff:(s + 1) * QW],
                                    KT[bp:bp + 64, ksl],
                                    QT[bp:bp + 64, qcols],
                                    start=True, stop=True,
                                    tile_position=(bp, 0),
                                )
                            ext = exp_pool.tile([P, 2 * QW], f32r,
                                                name="expt", tag="expt")
                            sco3 = sco[:].rearrange("p (t c) -> p t c", c=QW)
                            ext3 = ext[:].rearrange("p (t c) -> p t c", c=QW)
                            nc.scalar.activation(
                                ext3[:, :, coff:QW], sco3[:, :, coff:QW], EXP)
                            for s in range(2):
                                if 0 <= m <= 2:
                                    nc.vector.tensor_mul(
                                        ext[:, s * QW + coff:s * QW + coff + P],
                                        ext[:, s * QW + coff:s * QW + coff + P],
                                        tri_t[:],
                                    )
                                elif m == 3:
                                    nc.vector.tensor_mul(
                                        ext[:, s * QW + coff:(s + 1) * QW],
                                        ext[:, s * QW + coff:(s + 1) * QW],
                                        tri2_t[:],
                                    )
                                h = 2 * hp + s
                                nc.tensor.matmul(
                                    acc[s][:, coff:QW],
                                    vext[kj][:, h * P:(h + 1) * P],
                                    ext[:, s * QW + coff:(s + 1) * QW],
                                    start=(kj == 0), stop=(kj == nkj - 1),
                                    skip_group_check=True,
                                )
                        for s in range(2):
                            h = 2 * hp + s
                            rc = rcp_pool.tile([DH, QW], f32, name="rcp",
                                               tag="rcp")
                            nc.vector.reciprocal(rc[:], acc[s][DH:P, :])
                            dst = attnT[h // 2][
                                DH * (h % 2):DH * (h % 2) + DH,
                                qq * QW:(qq + 1) * QW,
                            ]
                            nc.vector.tensor_mul(dst, acc[s][0:DH, :], rc[:])

              # ---- phase 4: output projection -----------------------
              with (
                tc.tile_pool(name="wo", bufs=1) as wo_pool,
                tc.tile_pool(name="ops", bufs=2, space="PSUM") as o_psum,
                tc.tile_pool(name="osb", bufs=3) as o_sbuf,
              ):
                wo = [wo_pool.tile([P, D], f32r, name=f"wo{i}", tag=f"wo{i}")
                      for i in range(3)]
                for kd in range(3):
                    nc.sync.dma_start(wo[kd][:], wo_d[kd * P:(kd + 1) * P, :])
                for sm in range(SM):
                    ops = o_psum.tile([P, D], f32, name="ops", tag="ops")
                    ssl = slice(sm * P, (sm + 1) * P)
                    for kd in range(3):
                        nc.tensor.matmul(
                            ops[:, 0:512], attnT[kd][:, ssl],
                            wo[kd][:, 0:512],
                            start=(kd == 0), stop=(kd == 2),
                        )
                        nc.tensor.matmul(
                            ops[:, 512:D], attnT[kd][:, ssl],
                            wo[kd][:, 512:D],
                            start=(kd == 0), stop=(kd == 2),
                        )
                    osb = o_sbuf.tile([P, D], f32, name="osb", tag="osb")
                    nc.scalar.copy(osb[:], ops[:])
                    nc.sync.dma_start(out_d[ssl, :], osb[:])

    if split:
        _split_waits(nc)
    return nc


def _split_waits(nc, keep=1, per_nop=1):
    """Walrus TPB instructions support at most 1 sync wait + 1 update.
    Tile can emit more waits; spill the excess onto preceding
    same-engine NoOps (sequencer-level waits)."""
    import concourse.mybir as mybir

    n = 0
    for f in nc.m.functions:
        for bb in f.blocks:
            out = []
            changed = False
            for inst in bb.instructions:
                si = getattr(inst, "sync_info", None)
                if (
                    si is not None
                    and si.on_wait
                    and not isinstance(inst, mybir.InstEventSemaphore)
                ):
                    nu = len(si.on_update or [])
                    budget = max(0, min(keep, 2 - nu))
                    waits = list(si.on_wait)
                    if len(waits) > budget:
                        excess = waits[budget:]
                        while excess:
                            chunk, excess = excess[:per_nop], excess[per_nop:]
                            nop = mybir.InstNoOp(
                                name=f"wsplit_{n}", engine=inst.engine,
                                ins=[], outs=[])
                            n += 1
                            nop.sync_info = mybir.SyncInfo(
                                on_wait=chunk, on_update=[])
                            out.append(nop)
                        inst.sync_info = mybir.SyncInfo(
                            on_wait=waits[:budget],
                            on_update=list(si.on_update or []))
                        changed = True
                out.append(inst)
            if changed:
                bb.instructions = out
    return nc


_NC_CACHE = None


def get_nc():
    global _NC_CACHE
    if _NC_CACHE is None:
        _NC_CACHE = build_nc()
    return _NC_CACHE


def kernel(x, w_qkv, w_out):
    from concourse.bass_utils import run_bass_kernel_spmd

    in_maps = make_core_inputs(x, w_qkv, w_out)
    res = run_bass_kernel_spmd(get_nc(), in_maps, core_ids=list(range(NCORES)))
    out = np.empty((B, S, D), dtype=np.float32)
    for b in range(B):
        out[b] = res.results[2 * b]["out"] + res.results[2 * b + 1]["out"]
    return out
